# revision 23
# baseline (speedup 1.0000x reference)
"""GQA attention block (B=1, S=2048, HID=2048, NH=32, NKV=8, DH=64) on 8 trn2
NeuronCores.

Sharding: tensor-parallel over heads. Core c owns query heads [4c, 4c+4) and
KV head c (exactly one GQA group per core). Each core projects Q/K/V from the
full hidden states, applies RoPE, runs causal attention for its 4 heads, then
an AllToAll (split in two, so the first overlaps attention) re-shards the
attention output over sequence positions and each core computes the full
output projection for its 256 sequence rows. Host-side work is only
slicing/transposing/casting weights and concatenating the output shards.

Matmul inputs are bf16 (fp32 PSUM accumulation); softmax statistics are fp32
in PSUM. attention_mask is all-ones per the problem spec (fill "ones"); only
the causal mask is applied.
"""

import os
import sys

sys.path.insert(0, "/opt/trn_rl_repo")

import numpy as np
import ml_dtypes

import concourse.bacc as bacc
import concourse.mybir as mybir
import concourse.tile as tile
from concourse.bass_utils import run_bass_kernel_spmd

F32 = mybir.dt.float32
BF16 = mybir.dt.bfloat16
Exp = mybir.ActivationFunctionType.Exp

N_CORES = 8
S = 2048
HID = 2048
NH, NKV, DH = 32, 8, 64
NH_C = NH // N_CORES          # 4 query heads per core
P = 128
SC = 512                      # s-chunk (matmul free dim)
N_SC = S // SC                # 4
KT = HID // P                 # 16 contraction tiles
ST = S // P                   # 16 s-tiles of 128
SCALE = 1.0 / np.sqrt(DH)
SSH = S // N_CORES            # 256, sequence shard per core

last_results = None           # BassKernelResults of the most recent run


def _build():
    nc = bacc.Bacc("TRN2", target_bir_lowering=False, debug=False,
                   num_devices=N_CORES)

    # ---- kernel I/O ----
    hsp_d = nc.dram_tensor("hsp", [N_SC, P, KT * SC], BF16, kind="ExternalInput")
    wqp_d = nc.dram_tensor("wqp", [P, KT * NH_C * DH], BF16, kind="ExternalInput")
    wkvp_d = nc.dram_tensor("wkvp", [P, KT * 2 * DH], BF16, kind="ExternalInput")
    bv_d = nc.dram_tensor("bv", [P, 1], F32, kind="ExternalInput")
    cos_d = nc.dram_tensor("cos2", [P, S], F32, kind="ExternalInput")
    sin_d = nc.dram_tensor("sin2", [P, S], F32, kind="ExternalInput")
    rotw_d = nc.dram_tensor("rotw", [P, P], BF16, kind="ExternalInput")
    masks_d = nc.dram_tensor("masks", [P, 4, SC], BF16, kind="ExternalInput")
    identj_d = nc.dram_tensor("identj", [P, DH], BF16, kind="ExternalInput")
    vpad_d = nc.dram_tensor("vpad", [P, ST, DH], BF16, kind="ExternalInput")
    ones_d = nc.dram_tensor("ones", [P, P], BF16, kind="ExternalInput")
    wop_d = nc.dram_tensor("wop", [P, KT * HID], BF16, kind="ExternalInput")
    bo_d = nc.dram_tensor("bo", [1, HID], BF16, kind="ExternalInput")
    out_d = nc.dram_tensor("out", [SSH, HID], F32, kind="ExternalOutput")

    # internal DRAM for the sequence re-shard, one buffer per head-pair so
    # the first AllToAll can run while the second head-pair is computed
    a2a_in = [nc.dram_tensor(f"a2a_in{i}", [N_CORES, P, SSH], BF16)
              for i in range(2)]
    a2a_out = [nc.dram_tensor(f"a2a_out{i}", [N_CORES, P, SSH], BF16)
               for i in range(2)]

    with tile.TileContext(nc) as tc:
        with tc.tile_pool(name="persist", bufs=1) as persist:
            # ---- critical-path constants (host-prearranged, contiguous) ----
            wq_sb = persist.tile([P, KT, NH_C * DH], BF16)
            nc.sync.dma_start(wq_sb[:],
                              wqp_d.rearrange("p (kt m) -> p kt m", kt=KT))
            wkv_sb = persist.tile([P, KT, 2 * DH], BF16)
            nc.sync.dma_start(wkv_sb[:],
                              wkvp_d.rearrange("p (kt m) -> p kt m", kt=KT))
            cos_sb = persist.tile([P, S], F32)
            sin_sb = persist.tile([P, S], F32)
            rotw_sb = persist.tile([P, P], BF16)
            nc.sync.dma_start(rotw_sb[:], rotw_d[:])
            masks_sb = persist.tile([P, 4, SC], BF16)
            identj_sb = persist.tile([P, DH], BF16)
            ones_sb = persist.tile([P, P], BF16)
            bv_sb = persist.tile([P, 1], F32)
            nc.sync.dma_start(bv_sb[:], bv_d[:])
            bo_sb = persist.tile([1, HID], BF16)

            # ---- persistent activations ----
            qT_sb = persist.tile([P, 2, S], BF16)      # 4 heads, 2 per 128-row tile
            kT_sb = persist.tile([P, S], BF16)         # rows 0:64 = kT, 64:128 = dup
            vT_sb = persist.tile([P, S], BF16)         # rows 64:128 = vT
            v_aug = persist.tile([P, ST, P], BF16)     # [j, s-tile, ones+pad+v]

            # ================= QKV projection + RoPE, interleaved with
            # attention: the scheduler fills exp-wait PE gaps with the next
            # chunk's projection matmuls =================
            with tc.tile_pool(name="hs", bufs=2) as hs_pool, \
                 tc.tile_pool(name="proj_ps", bufs=1, space="PSUM") as proj_ps, \
                 tc.tile_pool(name="scr_ps", bufs=1, space="PSUM") as scr_ps, \
                 tc.tile_pool(name="sc_ps", bufs=1, space="PSUM") as sc_ps_pool, \
                 tc.tile_pool(name="ot_ps", bufs=1, space="PSUM") as ot_ps_pool, \
                 tc.tile_pool(name="rope", bufs=2) as rope_pool, \
                 tc.tile_pool(name="expa", bufs=4) as expa_pool, \
                 tc.tile_pool(name="norm", bufs=2) as norm_pool, \
                 tc.tile_pool(name="otsb", bufs=4) as ot_sb_pool:

                def emit_proj(sc):
                    ss = slice(sc * SC, (sc + 1) * SC)
                    hs_t = hs_pool.tile([P, KT, SC], BF16, tag="hs", name=f"hs_{sc}")
                    hsp_r = hsp_d[sc].rearrange("p (kt s) -> p kt s", kt=KT)
                    for k4 in range(4):
                        nc.sync.dma_start(hs_t[:, 4 * k4:4 * k4 + 4, :],
                                          hsp_r[:, 4 * k4:4 * k4 + 4, :])

                    ps_q = proj_ps.tile([P, 2, SC], F32, tag="projq",
                                        name=f"psq_{sc}")
                    ps_kv = proj_ps.tile([P, SC], F32, tag="projkv",
                                         name=f"pskv_{sc}")
                    for kt in range(KT):
                        st = kt == 0
                        sp = kt == KT - 1
                        nc.tensor.matmul(ps_q[:, 0, :], wq_sb[:, kt, 0:P],
                                         hs_t[:, kt, :], start=st, stop=sp)
                        nc.tensor.matmul(ps_q[:, 1, :], wq_sb[:, kt, P:2 * P],
                                         hs_t[:, kt, :], start=st, stop=sp)
                        nc.tensor.matmul(ps_kv, wkv_sb[:, kt, :],
                                         hs_t[:, kt, :], start=st, stop=sp)

                    if sc == 0:
                        # deferred constant loads: queued after the chunk-0
                        # activations, needed once rope/attention start
                        nc.sync.dma_start(cos_sb[:], cos_d[:])
                        nc.sync.dma_start(sin_sb[:], sin_d[:])
                        nc.sync.dma_start(identj_sb[:], identj_d[:])
                        nc.sync.dma_start(v_aug[:, :, 0:DH], vpad_d[:])
                        nc.sync.dma_start(masks_sb[:], masks_d[:])
                        nc.sync.dma_start(ones_sb[:], ones_d[:])
                        nc.sync.dma_start(bo_sb[:], bo_d[:])

                    # RoPE on q (two 128-row tiles = 4 heads)
                    for m in range(2):
                        qcos = rope_pool.tile([P, SC], F32, tag="qcos")
                        nc.vector.tensor_mul(qcos[:], ps_q[:, m, :], cos_sb[:, ss])
                        qraw = rope_pool.tile([P, SC], BF16, tag="qraw")
                        nc.vector.tensor_copy(qraw[:], ps_q[:, m, :])
                        rot = scr_ps.tile([P, SC], F32, tag="scr")
                        nc.tensor.matmul(rot, rotw_sb[:], qraw[:],
                                         start=True, stop=True)
                        qsin = rope_pool.tile([P, SC], F32, tag="qsin")
                        nc.vector.tensor_mul(qsin[:], rot[:], sin_sb[:, ss])
                        nc.vector.tensor_add(qT_sb[:, m, ss], qcos[:], qsin[:])

                    # RoPE on k (rows 0:64 of kv psum)
                    kcos = rope_pool.tile([DH, SC], F32, tag="kcos")
                    nc.vector.tensor_mul(kcos[:], ps_kv[0:DH, :], cos_sb[0:DH, ss])
                    kraw = rope_pool.tile([DH, SC], BF16, tag="kraw")
                    nc.vector.tensor_copy(kraw[:], ps_kv[0:DH, :])
                    krot = scr_ps.tile([DH, SC], F32, tag="scr")
                    nc.tensor.matmul(krot, rotw_sb[0:DH, 0:DH], kraw[:],
                                     start=True, stop=True)
                    ksin = rope_pool.tile([DH, SC], F32, tag="ksin")
                    nc.vector.tensor_mul(ksin[:], krot[:], sin_sb[0:DH, ss])
                    nc.vector.tensor_add(kT_sb[0:DH, ss], kcos[:], ksin[:])
                    # duplicate kT into rows 64:128 for row-group pairing
                    nc.sync.dma_start(kT_sb[DH:P, ss], kT_sb[0:DH, ss])

                    # v (+bias, cast bf16) lives at rows 64:128; transpose into
                    # natural [s, d] layout behind the ones/zero columns
                    nc.vector.tensor_scalar_add(vT_sb[DH:P, ss], ps_kv[DH:P, :],
                                                bv_sb[DH:P, :])
                    for k4 in range(SC // P):
                        g = sc * (SC // P) + k4
                        tp = scr_ps.tile([P, DH], BF16, tag="scr")
                        nc.tensor.transpose(tp, vT_sb[DH:P, g * P:(g + 1) * P],
                                            identj_sb[DH:P, :])
                        nc.vector.tensor_copy(v_aug[:, g, DH:P], tp[:])

                def emit_attn(hp, ic):
                    a2a_r = a2a_in[hp].rearrange("d p s -> p d s")
                    isl = slice(ic * SC, (ic + 1) * SC)
                    n_jt = 4 * (ic + 1)
                    ot = ot_ps_pool.tile([P, 2, SC], F32, tag="ot",
                                         name=f"ot_{hp}_{ic}")
                    for jt in range(n_jt):
                        jsl = slice(jt * P, (jt + 1) * P)
                        st = jt == 0
                        sp = jt == n_jt - 1
                        sc_t = sc_ps_pool.tile([P, 2, SC], F32, tag="sc")
                        nc.tensor.matmul(sc_t[:, 0, :], kT_sb[0:DH, jsl],
                                         qT_sb[0:DH, hp, isl],
                                         start=True, stop=True)
                        nc.tensor.matmul(sc_t[:, 1, :], kT_sb[DH:P, jsl],
                                         qT_sb[DH:P, hp, isl],
                                         start=True, stop=True)
                        ex = expa_pool.tile([P, 2, SC], BF16, tag="ex")
                        nc.scalar.activation(ex[:], sc_t[:], Exp,
                                             scale=float(SCALE))
                        if jt >= 4 * ic:
                            r = jt - 4 * ic
                            mask_b = masks_sb[:, r:r + 1, :].to_broadcast(
                                [P, 2, SC])
                            nc.vector.tensor_mul(ex[:], ex[:], mask_b)
                        nc.tensor.matmul(ot[:, 0, :], v_aug[:, jt, :],
                                         ex[:, 0, :], start=st, stop=sp)
                        nc.tensor.matmul(ot[:, 1, :], v_aug[:, jt, :],
                                         ex[:, 1, :], start=st, stop=sp)
                    # normalize by the ones-column sums and ship out
                    for half in range(2):
                        lrow = half * DH          # row within the head-pair
                        denom = norm_pool.tile([1, SC], F32, tag="denom")
                        nc.vector.tensor_copy(denom[:], ot[0:1, half, :])
                        recip = norm_pool.tile([1, SC], F32, tag="recip")
                        nc.vector.reciprocal_approx_fast(recip[:], denom[:])
                        recipb = norm_pool.tile([1, SC], BF16, tag="recipb")
                        nc.vector.tensor_copy(recipb[:], recip[:])
                        bc_ps = scr_ps.tile([P, SC], F32, tag="scr",
                                            name=f"bc_{hp}_{ic}_{half}")
                        nc.tensor.matmul(bc_ps[DH:P, :],
                                         ones_sb[0:1, 0:DH], recipb[:],
                                         start=True, stop=True)
                        bc_sb = norm_pool.tile([P, SC], F32, tag="bcsb")
                        nc.vector.tensor_copy(bc_sb[DH:P, :], bc_ps[DH:P, :])
                        ot_sb = ot_sb_pool.tile([P, SC], BF16, tag="otsb")
                        nc.vector.tensor_mul(ot_sb[DH:P, :], ot[DH:P, half, :],
                                             bc_sb[DH:P, :])
                        nc.sync.dma_start(
                            a2a_r[lrow:lrow + DH, 2 * ic:2 * ic + 2, :],
                            ot_sb[DH:P, :].rearrange("p (d s) -> p d s", d=2))

                for ic in range(N_SC):
                    emit_proj(ic)
                    emit_attn(0, ic)
                nc.gpsimd.collective_compute(
                    "AllToAll", mybir.AluOpType.bypass,
                    replica_groups=[list(range(N_CORES))],
                    ins=[a2a_in[0][:]], outs=[a2a_out[0][:]])
                for ic in range(N_SC):
                    emit_attn(1, ic)
                nc.gpsimd.collective_compute(
                    "AllToAll", mybir.AluOpType.bypass,
                    replica_groups=[list(range(N_CORES))],
                    ins=[a2a_in[1][:]], outs=[a2a_out[1][:]])

            # ================= output projection =================
            # global j-tile jt: core e = jt//2, head-pair = jt%2
            o_flat = [a2a_out[i].rearrange("e p s -> (e p) s") for i in range(2)]
            wop_r = wop_d.rearrange("p (jt n) -> p jt n", jt=KT)
            with tc.tile_pool(name="osb", bufs=4) as o_pool, \
                 tc.tile_pool(name="wo", bufs=6) as wo_pool, \
                 tc.tile_pool(name="out_ps", bufs=8, space="PSUM") as out_ps_pool, \
                 tc.tile_pool(name="outsb", bufs=3) as out_sb_pool:
                op_ps = [[out_ps_pool.tile([P, SC], F32, tag="op",
                                           name=f"op_{m}_{n4}")
                          for n4 in range(4)] for m in range(2)]
                jt_order = [2 * e for e in range(N_CORES)] + \
                           [2 * e + 1 for e in range(N_CORES)]
                for i, jt in enumerate(jt_order):
                    e, half = jt // 2, jt % 2
                    o_t = o_pool.tile([P, SSH], BF16, tag="o")
                    nc.sync.dma_start(o_t[:], o_flat[half][e * P:(e + 1) * P, :])
                    wo_t = wo_pool.tile([P, HID], BF16, tag="wo")
                    nc.sync.dma_start(wo_t[:], wop_r[:, jt, :])
                    for m in range(2):
                        for n4 in range(4):
                            nc.tensor.matmul(
                                op_ps[m][n4],
                                o_t[:, m * P:(m + 1) * P],
                                wo_t[:, n4 * SC:(n4 + 1) * SC],
                                start=(i == 0), stop=False)
                for m in range(2):
                    for n4 in range(4):
                        nsl = slice(n4 * SC, (n4 + 1) * SC)
                        nc.tensor.matmul(op_ps[m][n4], ones_sb[0:1, :],
                                         bo_sb[:, nsl], start=False,
                                         stop=True)
                        out_sb = out_sb_pool.tile([P, SC], F32, tag="outsb")
                        nc.vector.tensor_copy(out_sb[:], op_ps[m][n4])
                        nc.sync.dma_start(out_d[m * P:(m + 1) * P, nsl],
                                          out_sb[:])

    nc.compile()
    return nc


_cached_nc = None


def kernel(hidden_states, attention_mask, cos, sin, Wq, Wk, Wv, bv, Wo, bo):
    global _cached_nc, last_results
    hidden_states = np.asarray(hidden_states, dtype=np.float32)
    attention_mask = np.asarray(attention_mask)
    if not np.all(attention_mask == 1):
        raise NotImplementedError("kernel assumes an all-ones attention_mask")
    cos = np.asarray(cos, dtype=np.float32)
    sin = np.asarray(sin, dtype=np.float32)
    Wq = np.asarray(Wq, dtype=np.float32)
    Wk = np.asarray(Wk, dtype=np.float32)
    Wv = np.asarray(Wv, dtype=np.float32)
    bv = np.asarray(bv, dtype=np.float32)
    Wo = np.asarray(Wo, dtype=np.float32)
    bo = np.asarray(bo, dtype=np.float32)
    bf = ml_dtypes.bfloat16

    hsT = hidden_states[0].T.astype(bf)                           # [HID, S]
    # -> [sc, p, kt*s_chunk]: SBUF layout for each 512-column chunk
    hsp = np.ascontiguousarray(
        hsT.reshape(KT, P, N_SC, SC).transpose(2, 1, 0, 3).reshape(
            N_SC, P, KT * SC))
    cosT = np.ascontiguousarray(cos[0].T)                         # [DH, S]
    sinT = np.ascontiguousarray(sin[0].T)
    cos2 = np.concatenate([cosT, cosT], axis=0)                   # [128, S]
    sin2 = np.concatenate([sinT, sinT], axis=0)

    # rotate-half as a matmul: rot[d] = sign(d) * q[(d+32) % 64], per 64-block
    rotw = np.zeros((P, P), dtype=np.float32)
    for blk in (0, DH):
        for d in range(DH):
            partner = (d + DH // 2) % DH
            sign = -1.0 if d < DH // 2 else 1.0
            rotw[blk + partner, blk + d] = sign
    rotw = rotw.astype(bf)

    # causal masks for the 4 diagonal block offsets: keep j' <= i' - 128*r
    jj = np.arange(P)[:, None]
    ii = np.arange(SC)[None, :]
    masks = np.stack([(jj <= ii - P * r) for r in range(4)], axis=1)
    masks = masks.astype(bf)                                      # [128, 4, 512]

    identj = np.zeros((P, DH), dtype=bf)
    identj[DH:, :] = np.eye(DH, dtype=bf)
    vpad = np.zeros((P, ST, DH), dtype=bf)
    vpad[:, :, 0] = 1.0
    ones = np.ones((P, P), dtype=bf)
    woT = Wo.T.astype(bf)                                         # [NH*DH, HID]
    wop = np.ascontiguousarray(
        woT.reshape(KT, P, HID).transpose(1, 0, 2).reshape(P, KT * HID))
    bo_row = np.ascontiguousarray(bo.reshape(1, HID)).astype(bf)

    in_maps = []
    for c in range(N_CORES):
        wqT_c = Wq[c * NH_C * DH:(c + 1) * NH_C * DH].T.astype(bf)
        wqp_c = np.ascontiguousarray(
            wqT_c.reshape(KT, P, NH_C * DH).transpose(1, 0, 2).reshape(
                P, KT * NH_C * DH))
        wkv_c = np.concatenate([Wk[c * DH:(c + 1) * DH],
                                Wv[c * DH:(c + 1) * DH]], axis=0)
        wkvT_c = wkv_c.T.astype(bf)
        wkvp_c = np.ascontiguousarray(
            wkvT_c.reshape(KT, P, 2 * DH).transpose(1, 0, 2).reshape(
                P, KT * 2 * DH))
        bv_c = np.zeros((P, 1), dtype=np.float32)
        bv_c[DH:, 0] = bv[c * DH:(c + 1) * DH]
        in_maps.append({
            "hsp": hsp, "wqp": wqp_c, "wkvp": wkvp_c, "bv": bv_c,
            "cos2": cos2, "sin2": sin2, "rotw": rotw, "masks": masks,
            "identj": identj, "vpad": vpad, "ones": ones,
            "wop": wop, "bo": bo_row,
        })

    if _cached_nc is None:
        _cached_nc = _build()
    res = run_bass_kernel_spmd(_cached_nc, in_maps, list(range(N_CORES)))
    last_results = res
    if res.exec_time_ns is not None:
        print(f"HW exec time: {res.exec_time_ns} ns")

    out = np.concatenate([res.results[c]["out"] for c in range(N_CORES)],
                         axis=0)
    return out.reshape(1, S, HID).astype(np.float32)
